# revision 24
# baseline (speedup 1.0000x reference)
"""Grouped-GEMM MoE expert MLP kernel for 8 Trainium2 NeuronCores.

Problem: x [8, 2048, 1024] f32, per-group W1 [8, 4096, 1024], b1 [8, 4096],
W2 [8, 1024, 4096], b2 [8, 1024] (torch Linear convention, y = x @ W.T + b):
  h1 = xg @ W1.T + b1        (per group)
  h2 = h1 @ W2.T + b2
Expert-parallel: core i owns group i entirely — no collectives.

Formulation is fully transposed so every DMA is contiguous and biases land on
the partition axis:
  h1T[o, m]   = matmul(lhsT=W1T[h,o] tiles, rhs=xT[h,m] tiles)  + b1[o]
  outT[ho, m] = matmul(lhsT=W2T[o,ho] tiles, rhs=h1T[o,m] tiles) + b2[ho]
(out = lhsT.T @ rhs contracts the partition axis of both operands.)
Host pre-transposes x/W1/W2 per shard and un-transposes the output.

Matmuls run in bfloat16 with fp32 PSUM accumulation and f32 biases.
Measured HW cadence: bf16 matmul [128k x 512f] = 215.8 ns back-to-back vs
fp32r's 226.7 ns (fp32r pays ~32 extra cycles per instruction for the 4-byte
weight load) — 2048 matmuls/core -> ~442 us PE floor.  End-to-end rel err
(vs f32 reference) ~3e-3, dominated by bf16 input rounding.

Per-core loop structure: 2 m-chunks of 1024 tokens; inside, 8 o-chunks of 512.
GEMM1 for an o-chunk feeds SBUF bf16 tiles h1T; GEMM2 accumulates PSUM over an
o-PAIR (1024, 8 k-steps) then folds into an f32 SBUF accumulator (first pair
via ScalarE copy+bias, later pairs via VectorE add) to keep PSUM pressure at
4+4 banks. Weights are streamed per m-chunk (2 x 16.8 MB bf16), x and out once.

Startup: const-fp32 + memset-bf16 warmup matmuls start the HAM clock-throttle
release window (~3.6us of gapless PE activity -> 4/8 to 8/8 clocks) while the
cold fill runs; the cold fill leads with tiny pilot DMAs on all three
dma-capable queues (sync/scalar/gpsimd) to absorb the multi-us first-transfer
DMA spin-up, then round-robins (W1 k-tile, x k-tile) pairs across queues in
consumption order, and pair 0's GEMM1 runs as a single fused 1024-wide pass
(8 live PSUM groups) to halve the cold DMA demand rate.  Tail: the final
pair's GEMM2 runs ms-split with per-half output DMAs on alternating queues
(gpsimd excluded - its epilogue DRAIN waits on its last DMA completion), the
very last block as 2x128KB quarter-DMAs.
"""
import sys

sys.path.insert(0, "/opt/trn_rl_repo")

import numpy as np

import concourse.bass as bass  # noqa: F401  (bass import initializes mybir deps)
import concourse.mybir as mybir
import concourse.tile as tile
from concourse import bacc
from concourse.bass_utils import run_bass_kernel_spmd

NUM_GEMMS = 8
HIDDEN = 1024
INTER = 4096
M = 2048  # tokens per group

M_CHUNK = 1024  # tokens per chunk (2 chunks)
MS = 512        # matmul moving free dim (one PSUM bank)
O_CHUNK = 512   # GEMM1 / weight-DMA granularity along INTER
O_PAIR = 1024   # GEMM2 PSUM accumulation span along INTER (8 k-steps)

f32 = mybir.dt.float32
bf16 = mybir.dt.bfloat16

N_MC = M // M_CHUNK              # 2
N_PAIR = INTER // O_PAIR         # 4
N_MS = M_CHUNK // MS             # 2
KT1 = HIDDEN // 128              # 8 k-tiles for GEMM1
KT2 = O_PAIR // 128              # 8 k-tiles per GEMM2 psum group
N_OT = O_CHUNK // 128            # 4 o-tiles per o-chunk
N_HT = HIDDEN // 128             # 8 hout-tiles
N_WARM = 8                       # bf16 warmup matmuls (HAM clock ramp)

_NC_CACHE = None


def build_nc():
    """Build + compile the single-core program (same on all 8 cores)."""
    global _NC_CACHE
    if _NC_CACHE is not None:
        return _NC_CACHE

    nc = bacc.Bacc("TRN2", target_bir_lowering=False, debug=False, num_devices=8)
    xT = nc.dram_tensor("xT", [HIDDEN, M], bf16, kind="ExternalInput").ap()
    w1T = nc.dram_tensor("w1T", [HIDDEN, INTER], bf16, kind="ExternalInput").ap()
    b1 = nc.dram_tensor("b1", [128, INTER // 128], f32, kind="ExternalInput").ap()
    w2T = nc.dram_tensor("w2T", [INTER, HIDDEN], bf16, kind="ExternalInput").ap()
    b2 = nc.dram_tensor("b2", [128, HIDDEN // 128], f32, kind="ExternalInput").ap()
    outT = nc.dram_tensor("outT", [HIDDEN, M], f32, kind="ExternalOutput").ap()

    ID = mybir.ActivationFunctionType.Identity

    with tile.TileContext(nc) as tc:
        with (
            tc.tile_pool(name="cst", bufs=1) as cst,
            tc.tile_pool(name="xp", bufs=1) as xp,
            tc.tile_pool(name="hp", bufs=1) as hp,
            tc.tile_pool(name="w1p", bufs=3) as w1p,
            tc.tile_pool(name="w1c", bufs=1) as w1cp,
            tc.tile_pool(name="w2p", bufs=2) as w2p,
            tc.tile_pool(name="h1p", bufs=2) as h1p,
            tc.tile_pool(name="ps1", bufs=4, space="PSUM") as ps1,
            tc.tile_pool(name="ps2", bufs=4, space="PSUM") as ps2,
        ):
            # PE warmup while the first DMAs fill: starts the HAM clock
            # throttle release window (4/8 -> 8/8 after ~5.9us of gapless PE
            # activity) as early as possible.  First a const-operand fp32
            # matmul (available the moment the Tensor queue clears its
            # preamble ~7.3us, runs ~2.2us at the throttled clock), then a
            # few bf16 matmuls on DVE-memset tiles to bridge until the first
            # weight/x tiles land (~11us).  Real matmuls take over at half
            # clock for ~2us until the release at ~13.2us.
            ps_junk = ps1.tile([128, MS], f32, tag="ps1", name="ps1t")
            cwarm = nc.const_aps.scalar_like(1.0, ps_junk[:, :])
            cbr = cwarm.broadcast_to([128, MS])
            nc.tensor.matmul(ps_junk[:1, :], cwarm, cbr, start=True, stop=True)
            wwarm = cst.tile([128, 128], bf16)
            xwarm = cst.tile([128, 512], bf16)
            nc.vector.memset(wwarm[:, :], 0.01)
            nc.vector.memset(xwarm[:, :], 0.5)
            for _ in range(N_WARM):
                ps_junk = ps1.tile([128, MS], f32, tag="ps1", name="ps1t")
                nc.tensor.matmul(ps_junk[:, :], wwarm[:, :], xwarm[:, :],
                                 start=True, stop=True)

            b1_sb = cst.tile([128, INTER // 128], f32)
            b2_sb = cst.tile([128, HIDDEN // 128], f32)

            for mc in range(N_MC):
                m0 = mc * M_CHUNK
                # x chunk: [HIDDEN, M_CHUNK] -> [128, KT1 * M_CHUNK].
                # Split per k-tile so the first GEMM1 matmuls can start as
                # soon as k-tile 0 lands (subtile deps) instead of after the
                # whole 2.1 MB chunk.
                xt_sb = xp.tile([128, KT1 * M_CHUNK], bf16, tag="xt")
                if mc != 0:
                    # Non-first chunk: one bulk prefetch queued behind
                    # current work.
                    nc.sync.dma_start(
                        xt_sb[:, :].rearrange("p (a m) -> p a m", m=M_CHUNK),
                        xT[:, m0:m0 + M_CHUNK].rearrange(
                            "(a p) m -> p a m", p=128),
                    )
                # output accumulator: [HIDDEN, M_CHUNK] -> [128, N_HT * M_CHUNK]
                h2_sb = hp.tile([128, N_HT * M_CHUNK], f32, tag="h2")

                for pair in range(N_PAIR):
                    h1_half = []
                    w2_half = []
                    if mc == 0 and pair == 0:
                        # Fused cold pair: both o-chunks of pair 0 as ONE
                        # 1024-wide GEMM1 pass with 8 live PSUM groups (all
                        # 8 banks).  8 matmuls per k-step halve the cold
                        # DMA demand rate to ~220GB/s, matching what the
                        # freshly spun-up DMA engines can supply, so the
                        # k-outer pass runs nearly gapless from the start.
                        # Each queue leads with a TINY pilot transfer (the
                        # biases + a bias re-load) so the several-us
                        # first-transfer DMA spin-up is paid on 4-16KB
                        # instead of a tile the PE is about to need.  The
                        # (W1 k, x k ms0) tiles then round-robin across
                        # the three dma-capable queues in consumption
                        # order; the ms1 x half trails.
                        w1_sb = w1cp.tile([128, KT1 * O_PAIR], bf16,
                                          tag="w1c")

                        def w1c_args(k, half):
                            return (
                                w1_sb[:, k * O_PAIR + half * O_CHUNK:
                                      k * O_PAIR + (half + 1) * O_CHUNK],
                                w1T[k * 128:(k + 1) * 128,
                                    half * O_CHUNK:(half + 1) * O_CHUNK],
                            )

                        def xt_half(k, ms):
                            return (
                                xt_sb[:, k * M_CHUNK + ms * MS:
                                      k * M_CHUNK + (ms + 1) * MS],
                                xT[k * 128:(k + 1) * 128,
                                   m0 + ms * MS:m0 + (ms + 1) * MS],
                            )
                        pilot_sb = cst.tile([128, HIDDEN // 128], f32)
                        nc.sync.dma_start(b2_sb[:, :], b2[:, :])
                        nc.scalar.dma_start(b1_sb[:, :], b1[:, :])
                        nc.gpsimd.dma_start(pilot_sb[:, :], b2[:, :])
                        queues = [nc.gpsimd, nc.sync, nc.scalar]
                        need = []
                        for k in range(KT1):
                            need.append(w1c_args(k, 0))
                            need.append(xt_half(k, 0))
                            need.append(w1c_args(k, 1))
                        for k in range(KT1):
                            need.append(xt_half(k, 1))
                        for j, args in enumerate(need):
                            queues[j % 3].dma_start(*args)

                        h1_half = [
                            h1p.tile([128, N_OT * M_CHUNK], bf16, tag="h1",
                                     name="h1t")
                            for _ in range(2)
                        ]
                        for ms in range(N_MS):
                            accs = ([ps1.tile([128, MS], f32, tag="ps1",
                                              name="ps1t")
                                     for _ in range(N_OT)]
                                    + [ps2.tile([128, MS], f32, tag="ps2",
                                                name="ps2t")
                                       for _ in range(N_OT)])
                            for k in range(KT1):
                                for ot in range(2 * N_OT):
                                    nc.tensor.matmul(
                                        accs[ot][:, :],
                                        w1_sb[:, k * O_PAIR + ot * 128:
                                              k * O_PAIR + (ot + 1) * 128],
                                        xt_sb[:, k * M_CHUNK + ms * MS:
                                              k * M_CHUNK + (ms + 1) * MS],
                                        start=(k == 0),
                                        stop=(k == KT1 - 1),
                                    )
                            for ot in range(2 * N_OT):
                                nc.scalar.activation(
                                    h1_half[ot // N_OT][
                                        :, (ot % N_OT) * M_CHUNK + ms * MS:
                                        (ot % N_OT) * M_CHUNK + (ms + 1) * MS],
                                    accs[ot][:, :],
                                    ID,
                                    bias=b1_sb[:, ot:ot + 1],
                                    scale=1.0,
                                )
                        for half in range(2):
                            o0 = half * O_CHUNK
                            w2_sb = w2p.tile([128, N_OT * HIDDEN], bf16,
                                             tag="w2")
                            nc.sync.dma_start(
                                w2_sb[:, :].rearrange("p (a n) -> p a n",
                                                      n=HIDDEN),
                                w2T[o0:o0 + O_CHUNK, :].rearrange(
                                    "(a p) n -> p a n", p=128),
                            )
                            w2_half.append(w2_sb)
                    else:
                        for half in range(2):
                            oc = pair * 2 + half
                            o0 = oc * O_CHUNK
                            # W1T slice [HIDDEN, O_CHUNK] -> [128, KT1*O_CHUNK]
                            w1_sb = w1p.tile([128, KT1 * O_CHUNK], bf16,
                                             tag="w1")
                            nc.sync.dma_start(
                                w1_sb[:, :].rearrange("p (a o) -> p a o",
                                                      o=O_CHUNK),
                                w1T[:, o0:o0 + O_CHUNK].rearrange(
                                    "(a p) o -> p a o", p=128),
                            )

                            # GEMM1: h1T[o0:o0+512, m-chunk] in bf16
                            h1_sb = h1p.tile([128, N_OT * M_CHUNK], bf16,
                                             tag="h1")
                            for ot in range(N_OT):
                                accs = [ps1.tile([128, MS], f32, tag="ps1",
                                                 name="ps1t")
                                        for _ in range(N_MS)]
                                for k in range(KT1):
                                    lhsT = w1_sb[:, k * O_CHUNK + ot * 128:
                                                 k * O_CHUNK + (ot + 1) * 128]
                                    for ms in range(N_MS):
                                        nc.tensor.matmul(
                                            accs[ms][:, :],
                                            lhsT,
                                            xt_sb[:, k * M_CHUNK + ms * MS:
                                                  k * M_CHUNK + (ms + 1) * MS],
                                            start=(k == 0),
                                            stop=(k == KT1 - 1),
                                        )
                                for ms in range(N_MS):
                                    nc.scalar.activation(
                                        h1_sb[:, ot * M_CHUNK + ms * MS:
                                              ot * M_CHUNK + (ms + 1) * MS],
                                        accs[ms][:, :],
                                        ID,
                                        bias=b1_sb[:, oc * N_OT + ot:
                                                   oc * N_OT + ot + 1],
                                        scale=1.0,
                                    )
                            h1_half.append(h1_sb)

                            # W2T slice [O_CHUNK, HIDDEN] -> [128, N_OT*HIDDEN]
                            # Emitted after GEMM1 so its DMA queues behind
                            # the critical-path x/W1 loads.
                            w2_sb = w2p.tile([128, N_OT * HIDDEN], bf16,
                                             tag="w2")
                            nc.sync.dma_start(
                                w2_sb[:, :].rearrange("p (a n) -> p a n",
                                                      n=HIDDEN),
                                w2T[o0:o0 + O_CHUNK, :].rearrange(
                                    "(a p) n -> p a n", p=128),
                            )
                            w2_half.append(w2_sb)

                    # GEMM2 for the o-pair: accumulate 8 k-steps in PSUM,
                    # then fold into h2_sb.
                    # On the final pair of the final m-chunk the k-loop runs
                    # ms-split (all ms0 steps, fold, half-DMA, then ms1) so
                    # the last output transfers overlap the remaining
                    # matmuls and the tail after the last matmul is just one
                    # 256KB half-DMA + fold.
                    tail_pair = pair == N_PAIR - 1 and mc == N_MC - 1
                    for ht in range(N_HT):
                        last_ht = tail_pair and ht == N_HT - 1
                        n_acc = 1 if last_ht else N_MS
                        accs = [ps2.tile([128, MS], f32, tag="ps2", name="ps2t")
                                for _ in range(n_acc)]
                        ms_groups = ([[0, 1]] if not tail_pair
                                     else [[0]] if last_ht
                                     else [[0], [1]])
                        for ms_grp in ms_groups:
                            for k in range(KT2):
                                half, ot = divmod(k, N_OT)
                                lhsT = w2_half[half][:, ot * HIDDEN + ht * 128:
                                                     ot * HIDDEN + (ht + 1) * 128]
                                for ms in ms_grp:
                                    nc.tensor.matmul(
                                        accs[ms][:, :],
                                        lhsT,
                                        h1_half[half][:, ot * M_CHUNK + ms * MS:
                                                      ot * M_CHUNK + (ms + 1) * MS],
                                        start=(k == 0),
                                        stop=(k == KT2 - 1),
                                    )
                            for ms in ms_grp:
                                dst = h2_sb[:, ht * M_CHUNK + ms * MS:
                                            ht * M_CHUNK + (ms + 1) * MS]
                                if pair == 0:
                                    nc.scalar.activation(
                                        dst, accs[ms][:, :], ID,
                                        bias=b2_sb[:, ht:ht + 1], scale=1.0,
                                    )
                                else:
                                    nc.vector.tensor_add(dst, dst,
                                                         accs[ms][:, :])
                                if tail_pair:
                                    # per-half output DMA, alternating
                                    # queues; the very last block further
                                    # splits each half into two 128KB
                                    # quarter-DMAs on parallel queues.
                                    # gpsimd stays out of the tail: its
                                    # queue epilogue DRAIN is ~2.9us and
                                    # must not run after the last transfer.
                                    if ht == N_HT - 1:
                                        qs = ([nc.sync, nc.scalar] if ms == 0
                                              else [nc.scalar, nc.sync])
                                        for qi, q in enumerate(qs):
                                            c0 = ms * MS + qi * (MS // 2)
                                            q.dma_start(
                                                outT[ht * 128:(ht + 1) * 128,
                                                     m0 + c0:
                                                     m0 + c0 + MS // 2],
                                                h2_sb[:, ht * M_CHUNK + c0:
                                                      ht * M_CHUNK + c0
                                                      + MS // 2],
                                            )
                                    else:
                                        q = nc.sync if ms == 0 else nc.scalar
                                        q.dma_start(
                                            outT[ht * 128:(ht + 1) * 128,
                                                 m0 + ms * MS:
                                                 m0 + (ms + 1) * MS],
                                            h2_sb[:, ht * M_CHUNK + ms * MS:
                                                  ht * M_CHUNK
                                                  + (ms + 1) * MS],
                                        )
                        if last_ht:
                            # The very last ms1 half runs as TWO 256-wide
                            # PSUM groups in separate banks, so the first
                            # column-half's fold + 64KB DMA overlap the
                            # second half's matmuls; after the final matmul
                            # only a ~420ns half-fold + 64KB transfer
                            # remain.
                            HMS = MS // 2
                            for ci in range(2):
                                acc = ps2.tile([128, MS], f32, tag="ps2",
                                               name="ps2t")
                                c0 = MS + ci * HMS
                                for k in range(KT2):
                                    half, ot = divmod(k, N_OT)
                                    lhsT = w2_half[half][
                                        :, ot * HIDDEN + ht * 128:
                                        ot * HIDDEN + (ht + 1) * 128]
                                    nc.tensor.matmul(
                                        acc[:, 0:HMS],
                                        lhsT,
                                        h1_half[half][
                                            :, ot * M_CHUNK + c0:
                                            ot * M_CHUNK + c0 + HMS],
                                        start=(k == 0),
                                        stop=(k == KT2 - 1),
                                    )
                                dst = h2_sb[:, ht * M_CHUNK + c0:
                                            ht * M_CHUNK + c0 + HMS]
                                nc.vector.tensor_add(dst, dst, acc[:, 0:HMS])
                                q = nc.scalar if ci == 0 else nc.sync
                                q.dma_start(
                                    outT[ht * 128:(ht + 1) * 128,
                                         m0 + c0:m0 + c0 + HMS],
                                    dst,
                                )
                        if pair == N_PAIR - 1 and not tail_pair:
                            # Stream each hout-row-block out as soon as its
                            # last fold lands — keeps the kernel tail short.
                            nc.sync.dma_start(
                                outT[ht * 128:(ht + 1) * 128,
                                     m0:m0 + M_CHUNK],
                                h2_sb[:, ht * M_CHUNK:(ht + 1) * M_CHUNK],
                            )

    nc.compile()
    _NC_CACHE = nc
    return nc


def _prep_core_inputs(x, W1, b1, W2, b2, i):
    import ml_dtypes
    bf = ml_dtypes.bfloat16
    return {
        "xT": np.ascontiguousarray(
            np.asarray(x[i], dtype=np.float32).T).astype(bf),
        "w1T": np.ascontiguousarray(
            np.asarray(W1[i], dtype=np.float32).T).astype(bf),
        "b1": np.ascontiguousarray(
            np.asarray(b1[i], dtype=np.float32).reshape(INTER // 128, 128).T),
        "w2T": np.ascontiguousarray(
            np.asarray(W2[i], dtype=np.float32).T).astype(bf),
        "b2": np.ascontiguousarray(
            np.asarray(b2[i], dtype=np.float32).reshape(HIDDEN // 128, 128).T),
    }


def kernel(x, W1, b1, W2, b2, _trace=False, _trace_kwargs=None):
    x = np.asarray(x, dtype=np.float32)
    orig_shape = x.shape
    xg = x.reshape(NUM_GEMMS, M, HIDDEN)

    nc = build_nc()
    in_maps = [_prep_core_inputs(xg, W1, b1, W2, b2, i) for i in range(NUM_GEMMS)]
    res = None
    for attempt in range(3):
        try:
            res = run_bass_kernel_spmd(
                nc, in_maps, list(range(NUM_GEMMS)),
                trace=_trace, **(_trace_kwargs or {}),
            )
            break
        except Exception:
            # transient NRT_EXEC_UNIT_UNRECOVERABLE has been observed on
            # rapid repeated runs; a short pause and retry recovers
            if attempt == 2:
                raise
            import time
            time.sleep(20)
    out = np.stack(
        [res.results[i]["outT"].T for i in range(NUM_GEMMS)], axis=0
    ).reshape(orig_shape).astype(np.float32)
    if _trace:
        return out, res
    return out


# revision 25
# speedup vs baseline: 1.1946x; 1.1946x over previous
"""Grouped-GEMM MoE expert MLP kernel for 8 Trainium2 NeuronCores.

Problem: x [8, 2048, 1024] f32, per-group W1 [8, 4096, 1024], b1 [8, 4096],
W2 [8, 1024, 4096], b2 [8, 1024] (torch Linear convention, y = x @ W.T + b):
  h1 = xg @ W1.T + b1        (per group)
  h2 = h1 @ W2.T + b2
Expert-parallel: core i owns group i entirely — no collectives.

Formulation is fully transposed so every DMA is contiguous and biases land on
the partition axis:
  h1T[o, m]   = matmul(lhsT=W1T[h,o] tiles, rhs=xT[h,m] tiles)  + b1[o]
  outT[ho, m] = matmul(lhsT=W2T[o,ho] tiles, rhs=h1T[o,m] tiles) + b2[ho]
(out = lhsT.T @ rhs contracts the partition axis of both operands.)
Host pre-transposes x/W1/W2 per shard and un-transposes the output.

Matmuls run in bfloat16 with fp32 PSUM accumulation and f32 biases.
Measured HW cadence: bf16 matmul [128k x 512f] = 215.8 ns back-to-back vs
fp32r's 226.7 ns (fp32r pays ~32 extra cycles per instruction for the 4-byte
weight load) — 2048 matmuls/core -> ~442 us PE floor.  End-to-end rel err
(vs f32 reference) ~3e-3, dominated by bf16 input rounding.

Per-core loop structure: 2 m-chunks of 1024 tokens; inside, 8 o-chunks of 512.
GEMM1 for an o-chunk feeds SBUF bf16 tiles h1T; GEMM2 accumulates PSUM over an
o-PAIR (1024, 8 k-steps) then folds into an f32 SBUF accumulator (first pair
via ScalarE copy+bias, later pairs via VectorE add) to keep PSUM pressure at
4+4 banks. Weights are streamed per m-chunk (2 x 16.8 MB bf16), x and out once.

Startup: const-fp32 + memset-bf16 warmup matmuls start the HAM clock-throttle
release window (~3.6us of gapless PE activity -> 4/8 to 8/8 clocks) while the
cold fill runs; the cold fill leads with tiny pilot DMAs on all three
dma-capable queues (sync/scalar/gpsimd) to absorb the multi-us first-transfer
DMA spin-up, then round-robins (W1 k-tile, x k-tile) pairs across queues in
consumption order, and pair 0's GEMM1 runs as a single fused 1024-wide pass
(8 live PSUM groups) to halve the cold DMA demand rate.  Tail: the final
pair's GEMM2 runs ms-split with per-half output DMAs on alternating queues
(gpsimd excluded - its epilogue DRAIN waits on its last DMA completion), the
very last block as 2x128KB quarter-DMAs.
"""
import sys

sys.path.insert(0, "/opt/trn_rl_repo")

import numpy as np

import concourse.bass as bass  # noqa: F401  (bass import initializes mybir deps)
import concourse.mybir as mybir
import concourse.tile as tile
from concourse import bacc
from concourse.bass_utils import run_bass_kernel_spmd

NUM_GEMMS = 8
HIDDEN = 1024
INTER = 4096
M = 2048  # tokens per group

M_CHUNK = 1024  # tokens per chunk (2 chunks)
MS = 512        # matmul moving free dim (one PSUM bank)
O_CHUNK = 512   # GEMM1 / weight-DMA granularity along INTER
O_PAIR = 1024   # GEMM2 PSUM accumulation span along INTER (8 k-steps)

f32 = mybir.dt.float32
bf16 = mybir.dt.bfloat16

N_MC = M // M_CHUNK              # 2
N_PAIR = INTER // O_PAIR         # 4
N_MS = M_CHUNK // MS             # 2
KT1 = HIDDEN // 128              # 8 k-tiles for GEMM1
KT2 = O_PAIR // 128              # 8 k-tiles per GEMM2 psum group
N_OT = O_CHUNK // 128            # 4 o-tiles per o-chunk
N_HT = HIDDEN // 128             # 8 hout-tiles
N_WARM = 12                      # bf16 warmup matmuls (HAM clock ramp +
                                 # cold-DMA-landing coverage: traces show a
                                 # ~1.5us PE gap when warmups run out before
                                 # the round-robin cold fill catches up)

_NC_CACHE = None


def build_nc():
    """Build + compile the single-core program (same on all 8 cores)."""
    global _NC_CACHE
    if _NC_CACHE is not None:
        return _NC_CACHE

    nc = bacc.Bacc("TRN2", target_bir_lowering=False, debug=False, num_devices=8)
    xT = nc.dram_tensor("xT", [HIDDEN, M], bf16, kind="ExternalInput").ap()
    w1T = nc.dram_tensor("w1T", [HIDDEN, INTER], bf16, kind="ExternalInput").ap()
    b1 = nc.dram_tensor("b1", [128, INTER // 128], f32, kind="ExternalInput").ap()
    w2T = nc.dram_tensor("w2T", [INTER, HIDDEN], bf16, kind="ExternalInput").ap()
    b2 = nc.dram_tensor("b2", [128, HIDDEN // 128], f32, kind="ExternalInput").ap()
    outT = nc.dram_tensor("outT", [HIDDEN, M], f32, kind="ExternalOutput").ap()

    ID = mybir.ActivationFunctionType.Identity

    with tile.TileContext(nc) as tc:
        with (
            tc.tile_pool(name="cst", bufs=1) as cst,
            tc.tile_pool(name="xp", bufs=1) as xp,
            tc.tile_pool(name="hp", bufs=1) as hp,
            tc.tile_pool(name="w1p", bufs=3) as w1p,
            tc.tile_pool(name="w1c", bufs=1) as w1cp,
            tc.tile_pool(name="w2p", bufs=2) as w2p,
            tc.tile_pool(name="h1p", bufs=2) as h1p,
            tc.tile_pool(name="ps1", bufs=4, space="PSUM") as ps1,
            tc.tile_pool(name="ps2", bufs=4, space="PSUM") as ps2,
        ):
            # PE warmup while the first DMAs fill: starts the HAM clock
            # throttle release window (4/8 -> 8/8 after ~5.9us of gapless PE
            # activity) as early as possible.  First a const-operand fp32
            # matmul (available the moment the Tensor queue clears its
            # preamble ~7.3us, runs ~2.2us at the throttled clock), then a
            # few bf16 matmuls on DVE-memset tiles to bridge until the first
            # weight/x tiles land (~11us).  Real matmuls take over at half
            # clock for ~2us until the release at ~13.2us.
            ps_junk = ps1.tile([128, MS], f32, tag="ps1", name="ps1t")
            cwarm = nc.const_aps.scalar_like(1.0, ps_junk[:, :])
            cbr = cwarm.broadcast_to([128, MS])
            nc.tensor.matmul(ps_junk[:1, :], cwarm, cbr, start=True, stop=True)
            wwarm = cst.tile([128, 128], bf16)
            xwarm = cst.tile([128, 512], bf16)
            nc.vector.memset(wwarm[:, :], 0.01)
            nc.vector.memset(xwarm[:, :], 0.5)
            for _ in range(N_WARM):
                ps_junk = ps1.tile([128, MS], f32, tag="ps1", name="ps1t")
                nc.tensor.matmul(ps_junk[:, :], wwarm[:, :], xwarm[:, :],
                                 start=True, stop=True)

            b1_sb = cst.tile([128, INTER // 128], f32)
            b2_sb = cst.tile([128, HIDDEN // 128], f32)

            for mc in range(N_MC):
                m0 = mc * M_CHUNK
                # x chunk: [HIDDEN, M_CHUNK] -> [128, KT1 * M_CHUNK].
                # Split per k-tile so the first GEMM1 matmuls can start as
                # soon as k-tile 0 lands (subtile deps) instead of after the
                # whole 2.1 MB chunk.
                xt_sb = xp.tile([128, KT1 * M_CHUNK], bf16, tag="xt")
                if mc != 0:
                    # Non-first chunk: one bulk prefetch queued behind
                    # current work.
                    nc.sync.dma_start(
                        xt_sb[:, :].rearrange("p (a m) -> p a m", m=M_CHUNK),
                        xT[:, m0:m0 + M_CHUNK].rearrange(
                            "(a p) m -> p a m", p=128),
                    )
                # output accumulator: [HIDDEN, M_CHUNK] -> [128, N_HT * M_CHUNK]
                h2_sb = hp.tile([128, N_HT * M_CHUNK], f32, tag="h2")

                for pair in range(N_PAIR):
                    h1_half = []
                    w2_half = []
                    if mc == 0 and pair == 0:
                        # Fused cold pair: both o-chunks of pair 0 as ONE
                        # 1024-wide GEMM1 pass with 8 live PSUM groups (all
                        # 8 banks).  8 matmuls per k-step halve the cold
                        # DMA demand rate to ~220GB/s, matching what the
                        # freshly spun-up DMA engines can supply, so the
                        # k-outer pass runs nearly gapless from the start.
                        # Each queue leads with a TINY pilot transfer (the
                        # biases + a bias re-load) so the several-us
                        # first-transfer DMA spin-up is paid on 4-16KB
                        # instead of a tile the PE is about to need.  The
                        # (W1 k, x k ms0) tiles then round-robin across
                        # the three dma-capable queues in consumption
                        # order; the ms1 x half trails.
                        w1_sb = w1cp.tile([128, KT1 * O_PAIR], bf16,
                                          tag="w1c")

                        def w1c_args(k, half):
                            return (
                                w1_sb[:, k * O_PAIR + half * O_CHUNK:
                                      k * O_PAIR + (half + 1) * O_CHUNK],
                                w1T[k * 128:(k + 1) * 128,
                                    half * O_CHUNK:(half + 1) * O_CHUNK],
                            )

                        def xt_half(k, ms):
                            return (
                                xt_sb[:, k * M_CHUNK + ms * MS:
                                      k * M_CHUNK + (ms + 1) * MS],
                                xT[k * 128:(k + 1) * 128,
                                   m0 + ms * MS:m0 + (ms + 1) * MS],
                            )
                        pilot_sb = cst.tile([128, HIDDEN // 128], f32)
                        nc.sync.dma_start(b2_sb[:, :], b2[:, :])
                        nc.scalar.dma_start(b1_sb[:, :], b1[:, :])
                        nc.gpsimd.dma_start(pilot_sb[:, :], b2[:, :])
                        queues = [nc.gpsimd, nc.sync, nc.scalar]
                        need = []
                        for k in range(KT1):
                            need.append(w1c_args(k, 0))
                            need.append(xt_half(k, 0))
                            need.append(w1c_args(k, 1))
                        for k in range(KT1):
                            need.append(xt_half(k, 1))
                        for j, args in enumerate(need):
                            queues[j % 3].dma_start(*args)

                        h1_half = [
                            h1p.tile([128, N_OT * M_CHUNK], bf16, tag="h1",
                                     name="h1t")
                            for _ in range(2)
                        ]
                        for ms in range(N_MS):
                            accs = ([ps1.tile([128, MS], f32, tag="ps1",
                                              name="ps1t")
                                     for _ in range(N_OT)]
                                    + [ps2.tile([128, MS], f32, tag="ps2",
                                                name="ps2t")
                                       for _ in range(N_OT)])
                            for k in range(KT1):
                                for ot in range(2 * N_OT):
                                    nc.tensor.matmul(
                                        accs[ot][:, :],
                                        w1_sb[:, k * O_PAIR + ot * 128:
                                              k * O_PAIR + (ot + 1) * 128],
                                        xt_sb[:, k * M_CHUNK + ms * MS:
                                              k * M_CHUNK + (ms + 1) * MS],
                                        start=(k == 0),
                                        stop=(k == KT1 - 1),
                                    )
                            for ot in range(2 * N_OT):
                                nc.scalar.activation(
                                    h1_half[ot // N_OT][
                                        :, (ot % N_OT) * M_CHUNK + ms * MS:
                                        (ot % N_OT) * M_CHUNK + (ms + 1) * MS],
                                    accs[ot][:, :],
                                    ID,
                                    bias=b1_sb[:, ot:ot + 1],
                                    scale=1.0,
                                )
                        for half in range(2):
                            o0 = half * O_CHUNK
                            w2_sb = w2p.tile([128, N_OT * HIDDEN], bf16,
                                             tag="w2")
                            nc.sync.dma_start(
                                w2_sb[:, :].rearrange("p (a n) -> p a n",
                                                      n=HIDDEN),
                                w2T[o0:o0 + O_CHUNK, :].rearrange(
                                    "(a p) n -> p a n", p=128),
                            )
                            w2_half.append(w2_sb)
                    else:
                        for half in range(2):
                            oc = pair * 2 + half
                            o0 = oc * O_CHUNK
                            # W1T slice [HIDDEN, O_CHUNK] -> [128, KT1*O_CHUNK]
                            w1_sb = w1p.tile([128, KT1 * O_CHUNK], bf16,
                                             tag="w1")
                            nc.sync.dma_start(
                                w1_sb[:, :].rearrange("p (a o) -> p a o",
                                                      o=O_CHUNK),
                                w1T[:, o0:o0 + O_CHUNK].rearrange(
                                    "(a p) o -> p a o", p=128),
                            )

                            # GEMM1: h1T[o0:o0+512, m-chunk] in bf16
                            h1_sb = h1p.tile([128, N_OT * M_CHUNK], bf16,
                                             tag="h1")
                            for ot in range(N_OT):
                                accs = [ps1.tile([128, MS], f32, tag="ps1",
                                                 name="ps1t")
                                        for _ in range(N_MS)]
                                for k in range(KT1):
                                    lhsT = w1_sb[:, k * O_CHUNK + ot * 128:
                                                 k * O_CHUNK + (ot + 1) * 128]
                                    for ms in range(N_MS):
                                        nc.tensor.matmul(
                                            accs[ms][:, :],
                                            lhsT,
                                            xt_sb[:, k * M_CHUNK + ms * MS:
                                                  k * M_CHUNK + (ms + 1) * MS],
                                            start=(k == 0),
                                            stop=(k == KT1 - 1),
                                        )
                                for ms in range(N_MS):
                                    nc.scalar.activation(
                                        h1_sb[:, ot * M_CHUNK + ms * MS:
                                              ot * M_CHUNK + (ms + 1) * MS],
                                        accs[ms][:, :],
                                        ID,
                                        bias=b1_sb[:, oc * N_OT + ot:
                                                   oc * N_OT + ot + 1],
                                        scale=1.0,
                                    )
                            h1_half.append(h1_sb)

                            # W2T slice [O_CHUNK, HIDDEN] -> [128, N_OT*HIDDEN]
                            # Emitted after GEMM1 so its DMA queues behind
                            # the critical-path x/W1 loads.
                            w2_sb = w2p.tile([128, N_OT * HIDDEN], bf16,
                                             tag="w2")
                            nc.sync.dma_start(
                                w2_sb[:, :].rearrange("p (a n) -> p a n",
                                                      n=HIDDEN),
                                w2T[o0:o0 + O_CHUNK, :].rearrange(
                                    "(a p) n -> p a n", p=128),
                            )
                            w2_half.append(w2_sb)

                    # GEMM2 for the o-pair: accumulate 8 k-steps in PSUM,
                    # then fold into h2_sb.
                    # On the final pair of the final m-chunk the k-loop runs
                    # ms-split (all ms0 steps, fold, half-DMA, then ms1) so
                    # the last output transfers overlap the remaining
                    # matmuls and the tail after the last matmul is just one
                    # 256KB half-DMA + fold.
                    tail_pair = pair == N_PAIR - 1 and mc == N_MC - 1
                    for ht in range(N_HT):
                        last_ht = tail_pair and ht == N_HT - 1
                        n_acc = 1 if last_ht else N_MS
                        accs = [ps2.tile([128, MS], f32, tag="ps2", name="ps2t")
                                for _ in range(n_acc)]
                        ms_groups = ([[0, 1]] if not tail_pair
                                     else [[0]] if last_ht
                                     else [[0], [1]])
                        for ms_grp in ms_groups:
                            for k in range(KT2):
                                half, ot = divmod(k, N_OT)
                                lhsT = w2_half[half][:, ot * HIDDEN + ht * 128:
                                                     ot * HIDDEN + (ht + 1) * 128]
                                for ms in ms_grp:
                                    nc.tensor.matmul(
                                        accs[ms][:, :],
                                        lhsT,
                                        h1_half[half][:, ot * M_CHUNK + ms * MS:
                                                      ot * M_CHUNK + (ms + 1) * MS],
                                        start=(k == 0),
                                        stop=(k == KT2 - 1),
                                    )
                            for ms in ms_grp:
                                dst = h2_sb[:, ht * M_CHUNK + ms * MS:
                                            ht * M_CHUNK + (ms + 1) * MS]
                                if pair == 0:
                                    nc.scalar.activation(
                                        dst, accs[ms][:, :], ID,
                                        bias=b2_sb[:, ht:ht + 1], scale=1.0,
                                    )
                                else:
                                    nc.vector.tensor_add(dst, dst,
                                                         accs[ms][:, :])
                                if tail_pair:
                                    # per-half output DMA, alternating
                                    # queues; the very last block further
                                    # splits each half into two 128KB
                                    # quarter-DMAs on parallel queues.
                                    # gpsimd stays out of the tail: its
                                    # queue epilogue DRAIN is ~2.9us and
                                    # must not run after the last transfer.
                                    if ht == N_HT - 1:
                                        qs = ([nc.sync, nc.scalar] if ms == 0
                                              else [nc.scalar, nc.sync])
                                        for qi, q in enumerate(qs):
                                            c0 = ms * MS + qi * (MS // 2)
                                            q.dma_start(
                                                outT[ht * 128:(ht + 1) * 128,
                                                     m0 + c0:
                                                     m0 + c0 + MS // 2],
                                                h2_sb[:, ht * M_CHUNK + c0:
                                                      ht * M_CHUNK + c0
                                                      + MS // 2],
                                            )
                                    else:
                                        q = nc.sync if ms == 0 else nc.scalar
                                        q.dma_start(
                                            outT[ht * 128:(ht + 1) * 128,
                                                 m0 + ms * MS:
                                                 m0 + (ms + 1) * MS],
                                            h2_sb[:, ht * M_CHUNK + ms * MS:
                                                  ht * M_CHUNK
                                                  + (ms + 1) * MS],
                                        )
                        if last_ht:
                            # The very last ms1 half runs as TWO 256-wide
                            # PSUM groups in separate banks, so the first
                            # column-half's fold + 64KB DMA overlap the
                            # second half's matmuls; after the final matmul
                            # only a ~420ns half-fold + 64KB transfer
                            # remain.
                            HMS = MS // 2
                            for ci in range(2):
                                acc = ps2.tile([128, MS], f32, tag="ps2",
                                               name="ps2t")
                                c0 = MS + ci * HMS
                                for k in range(KT2):
                                    half, ot = divmod(k, N_OT)
                                    lhsT = w2_half[half][
                                        :, ot * HIDDEN + ht * 128:
                                        ot * HIDDEN + (ht + 1) * 128]
                                    nc.tensor.matmul(
                                        acc[:, 0:HMS],
                                        lhsT,
                                        h1_half[half][
                                            :, ot * M_CHUNK + c0:
                                            ot * M_CHUNK + c0 + HMS],
                                        start=(k == 0),
                                        stop=(k == KT2 - 1),
                                    )
                                dst = h2_sb[:, ht * M_CHUNK + c0:
                                            ht * M_CHUNK + c0 + HMS]
                                nc.vector.tensor_add(dst, dst, acc[:, 0:HMS])
                                q = nc.scalar if ci == 0 else nc.sync
                                q.dma_start(
                                    outT[ht * 128:(ht + 1) * 128,
                                         m0 + c0:m0 + c0 + HMS],
                                    dst,
                                )
                        if pair == N_PAIR - 1 and not tail_pair:
                            # Stream each hout-row-block out as soon as its
                            # last fold lands — keeps the kernel tail short.
                            nc.sync.dma_start(
                                outT[ht * 128:(ht + 1) * 128,
                                     m0:m0 + M_CHUNK],
                                h2_sb[:, ht * M_CHUNK:(ht + 1) * M_CHUNK],
                            )

    nc.compile()
    _NC_CACHE = nc
    return nc


def _prep_core_inputs(x, W1, b1, W2, b2, i):
    import ml_dtypes
    bf = ml_dtypes.bfloat16
    return {
        "xT": np.ascontiguousarray(
            np.asarray(x[i], dtype=np.float32).T).astype(bf),
        "w1T": np.ascontiguousarray(
            np.asarray(W1[i], dtype=np.float32).T).astype(bf),
        "b1": np.ascontiguousarray(
            np.asarray(b1[i], dtype=np.float32).reshape(INTER // 128, 128).T),
        "w2T": np.ascontiguousarray(
            np.asarray(W2[i], dtype=np.float32).T).astype(bf),
        "b2": np.ascontiguousarray(
            np.asarray(b2[i], dtype=np.float32).reshape(HIDDEN // 128, 128).T),
    }


def kernel(x, W1, b1, W2, b2, _trace=False, _trace_kwargs=None):
    x = np.asarray(x, dtype=np.float32)
    orig_shape = x.shape
    xg = x.reshape(NUM_GEMMS, M, HIDDEN)

    nc = build_nc()
    in_maps = [_prep_core_inputs(xg, W1, b1, W2, b2, i) for i in range(NUM_GEMMS)]
    res = None
    for attempt in range(3):
        try:
            res = run_bass_kernel_spmd(
                nc, in_maps, list(range(NUM_GEMMS)),
                trace=_trace, **(_trace_kwargs or {}),
            )
            break
        except Exception:
            # transient NRT_EXEC_UNIT_UNRECOVERABLE has been observed on
            # rapid repeated runs; a short pause and retry recovers
            if attempt == 2:
                raise
            import time
            time.sleep(20)
    out = np.stack(
        [res.results[i]["outT"].T for i in range(NUM_GEMMS)], axis=0
    ).reshape(orig_shape).astype(np.float32)
    if _trace:
        return out, res
    return out
